# revision 1
# baseline (speedup 1.0000x reference)
"""CompressedLinear (quantized low-rank linear) on 8 trn2 NeuronCores.

y = ((x @ dequant(Vh).T) * dequant(S)) @ dequant(U).T + bias

Strategy: data-parallel over tokens. x [4,2048,4096] -> 8192 tokens -> 1024
tokens/core. Weights replicated. Per core, two chained bf16 matmuls with
fp32 PSUM accumulation:

  mm1: hT[r, tok]  = (Vh_int - zp_v).T-tile.T @ xT-tile   (contract in_f)
  mm2: y[tok, o]   = hT-tile.T @ (U_int - zp_u).T-tile    (contract rank)

All multiplicative scales (Vh_scale * S_scale * U_scale) and the dequantized
S vector are folded into the hT eviction (per-partition scalar on rank), so
the integer-valued weights stay EXACT in bf16 (-128..127 fits in 8-bit
mantissa); the only bf16 rounding is on x, hT and the final y store.

Perf notes (from trace analysis of the v1 kernel):
 * DMA trigger instructions cost ~600-800ns each and serialize per engine
   queue, so the startup-critical loads are spread across otherwise-idle
   queues: weights on Sync, x on GpSimd, consts on Vector. The first vh
   tile is split in half (with split zero-point subtracts on DVE/ACT) so
   the first LDWEIGHTS dependency is ~1KB/partition of DMA, not 2KB.
 * The PE clock starts gated at 1.2GHz and only reaches 2.4GHz after
   ~3.4us of sustained activity (HAM). A handful of dummy matmuls on a
   zeroed tile run while the first loads are in flight, so the ramp is
   paid during otherwise-idle time.
 * y is stored as bf16 (host upcasts to fp32; ~1e-3 extra rel err) and
   evictions alternate DVE/GpSimd with DMA triggers alternating
   Scalar/Sync; the very last tile is split in half across two engine
   pairs so the end-of-kernel drain is short.

Host does pure layout work only: x transpose/shard, weight transpose and a
lossless int32->bf16 cast (values are 0..255, exact in bf16). All
arithmetic (zero-point subtract, scales, matmuls, bias) runs on device.
"""

import os

import numpy as np
import ml_dtypes

IN_F, OUT_F, RANK = 4096, 4096, 1024
B, S_LEN = 4, 2048
N_CORES = 8
P = 128
NTOK = B * S_LEN            # 8192 tokens total
TPC = NTOK // N_CORES       # 1024 tokens per core
TBS = 512                   # tokens per block (matmul moving free dim)
TB = TPC // TBS             # 2 token blocks per core
KO = IN_F // P              # 32 k-tiles (mm1 contraction)
RO = RANK // P              # 8 r-tiles (mm2 contraction / mm1 output)
NOB = OUT_F // 512          # 8 output-feature blocks of 512
NWARM = 5                   # dummy matmuls to warm the PE clock gate

_BF16 = ml_dtypes.bfloat16

# Set by kernel() for the benefit of test harnesses (exec time inspection).
last_run = None

# Compiled-module cache: the NEFF only depends on the (scalar) quantization
# parameters, so repeat kernel() calls skip the rebuild.
_nc_cache = {}


def _build_nc(zp_v: float, zp_u: float, zp_s: float, s_mult: float):
    import concourse.mybir as mybir
    import concourse.tile as tile
    from concourse import bacc

    f32 = mybir.dt.float32
    bf16 = mybir.dt.bfloat16
    OP = mybir.AluOpType
    ACTF = mybir.ActivationFunctionType

    nc = bacc.Bacc("TRN2", target_bir_lowering=False, debug=False,
                   num_devices=N_CORES)

    # x repacked on host to [blk, ko4, 128, 4*512] so each partition row is a
    # 4 KiB contiguous DMA line (k = ko4*512 + four*128 + p, tokens inner).
    u8 = mybir.dt.uint8
    xr = nc.dram_tensor("xr", [TB, KO // 4, P, 4 * TBS], bf16,
                        kind="ExternalInput")
    vhT = nc.dram_tensor("vhT", [IN_F, RANK], u8, kind="ExternalInput")
    uT = nc.dram_tensor("uT", [RANK, OUT_F], u8, kind="ExternalInput")
    sv = nc.dram_tensor("sv", [RANK], bf16, kind="ExternalInput")
    bias = nc.dram_tensor("bias", [OUT_F], f32, kind="ExternalInput")
    y = nc.dram_tensor("y", [TPC, OUT_F], bf16, kind="ExternalOutput")

    with tile.TileContext(nc) as tc:
        with (
            tc.tile_pool(name="const", bufs=1) as const,
            tc.tile_pool(name="xbp", bufs=KO // 4) as xbp,
            tc.tile_pool(name="vstg", bufs=8) as vstg,
            tc.tile_pool(name="ustg", bufs=8) as ustg,
            tc.tile_pool(name="hTp", bufs=1) as hTp,
            tc.tile_pool(name="yout", bufs=4) as yout,
            tc.tile_pool(name="psp", bufs=8, space="PSUM") as psp,
        ):
            # ---- PE warm-up: dummy matmuls on a zeroed tile keep the PE
            # busy (and ramp its clock gate to 2.4GHz) while the first
            # input DMAs are still in flight. The scratch PSUM result is
            # never read; its pool slot recycles under mm1's 8-bank group.
            warm = const.tile([P, TBS], bf16, name="warm")
            nc.vector.memset(warm[:], 0.0)
            ps_w = psp.tile([P, TBS], f32, tag="ps", name="ps_warm")
            for _ in range(NWARM):
                nc.tensor.matmul(ps_w[:], warm[:, :P], warm[:],
                                 start=True, stop=True)

            vh_src = vhT.ap().rearrange("(ko p) r -> p ko r", p=P)
            u_src = uT.ap().rearrange("(ro p) o -> p ro o", p=P)

            # ---- Scalar queue: s_sb early (tiny) so s_comb is ready well
            # before the first hT eviction.
            # S vector -> folded per-rank scale: (S - zp_s) * (s_v*s_s*s_u)
            s_sb = const.tile([P, RO], bf16, name="s_sb")
            nc.scalar.dma_start(s_sb[:],
                                sv.ap().rearrange("(ro p) -> p ro", p=P))
            s_comb = const.tile([P, RO], f32, name="s_comb")
            nc.vector.tensor_scalar(s_comb[:], s_sb[:], zp_s, s_mult,
                                    OP.subtract, OP.mult)

            # ---- Single input stream on the Sync queue (one queue gets
            # the full ~300GB/s; splitting across engine queues just
            # steals bandwidth from the critical stream). Both weights
            # ride as uint8 (half the wire bytes; 0..255 exact) — the
            # zero-point subtract doubles as the u8->bf16 upconvert.
            # Order: vh + x block0 interleaved per k-group (consumed
            # immediately by mm1), then U, then bias, then x block1.
            # Staging pools are 8 deep so DMA triggers never chain on
            # subtract completions.
            vh_t = [None] * KO
            xq0 = []
            for ko4 in range(KO // 4):
                xb = xbp.tile([P, 4 * TBS], bf16, name="xb")
                def _vh_load(ko):
                    vt = const.tile([P, RANK], bf16, name=f"vh_{ko}")
                    vs = vstg.tile([P, RANK], u8, name="vh_stg")
                    nc.sync.dma_start(vs[:], vh_src[:, ko, :])
                    nc.vector.tensor_scalar(vt[:], vs[:], zp_v, None,
                                            OP.subtract)
                    vh_t[ko] = vt

                if ko4 == 0:
                    # vh0 first half, then the first x quarter, lead the
                    # queue so the first matmul's critical path is short;
                    # vh1-3 come before the bulk x so the k-loop is never
                    # paced by trigger-issue serialization.
                    vt = const.tile([P, RANK], bf16, name="vh_0")
                    vs = vstg.tile([P, RANK], u8, name="vh_stg")
                    h = RANK // 2
                    nc.sync.dma_start(vs[:, 0:h], vh_src[:, 0, 0:h])
                    nc.vector.tensor_scalar(vt[:, 0:h], vs[:, 0:h], zp_v,
                                            None, OP.subtract)
                    nc.sync.dma_start(xb[:, 0:TBS], xr.ap()[0, 0, :, 0:TBS])
                    nc.sync.dma_start(vs[:, h:], vh_src[:, 0, h:])
                    nc.vector.tensor_scalar(vt[:, h:], vs[:, h:], zp_v,
                                            None, OP.subtract)
                    vh_t[0] = vt
                    _vh_load(1)
                    nc.sync.dma_start(xb[:, TBS:], xr.ap()[0, 0, :, TBS:])
                    _vh_load(2)
                    _vh_load(3)
                else:
                    nc.sync.dma_start(xb[:], xr.ap()[0, ko4, :, :])
                    for j in range(4):
                        _vh_load(ko4 * 4 + j)
                xq0.append(xb)

            # U as uint8 in [128,1024] chunks; upconvert-subtracts
            # alternate DVE/ACT so neither engine queues deep enough to
            # delay the mm1->mm2 boundary evictions behind them.
            u_t = []
            for ro in range(RO):
                ut = const.tile([P, OUT_F], bf16, name=f"u_{ro}")
                for c in range(4):
                    us = ustg.tile([P, RANK], u8, name="u_stg")
                    sl = slice(c * RANK, (c + 1) * RANK)
                    nc.sync.dma_start(us[:], u_src[:, ro, sl])
                    if (ro * 4 + c) % 2:
                        nc.scalar.activation(ut[:, sl], us[:], ACTF.Copy,
                                             bias=-zp_u)
                    else:
                        nc.vector.tensor_scalar(ut[:, sl], us[:], zp_u,
                                                None, OP.subtract)
                u_t.append(ut)

            bias_sb = const.tile([P, OUT_F], f32, name="bias_sb")
            nc.sync.dma_start(bias_sb[:],
                              bias.ap()[None, :].to_broadcast((P, OUT_F)))

            for blk in range(TB):
                tok0 = blk * TBS
                if blk > 0:
                    xq0 = []
                    for ko4 in range(KO // 4):
                        xb = xbp.tile([P, 4 * TBS], bf16, name="xb")
                        nc.sync.dma_start(xb[:], xr.ap()[blk, ko4, :, :])
                        xq0.append(xb)

                # ---- mm1: hT[r, tok], all 8 r-tiles in one pass over ko so
                # each vh/x tile is consumed exactly once (8 PSUM banks) ----
                hT = hTp.tile([P, RO, TBS], bf16, name="hT")
                pst = [psp.tile([P, TBS], f32, tag="ps", name=f"ps1_{rt}")
                       for rt in range(RO)]
                for ko in range(KO):
                    rhs = xq0[ko // 4][:, (ko % 4) * TBS:(ko % 4 + 1) * TBS]
                    for rt in range(RO):
                        nc.tensor.matmul(
                            pst[rt][:],
                            vh_t[ko][:, rt * P:(rt + 1) * P],
                            rhs,
                            start=(ko == 0), stop=(ko == KO - 1))
                for rt in range(RO):
                    # hT = psum * s_comb[r] (per-partition scalar); alternate
                    # DVE/ScalarE so the evict chain at the mm1->mm2 boundary
                    # runs on two engines concurrently
                    if rt % 2 == 0:
                        nc.vector.tensor_tensor(
                            hT[:, rt, :], pst[rt][:],
                            s_comb[:, rt:rt + 1].to_broadcast((P, TBS)),
                            OP.mult)
                    else:
                        nc.scalar.activation(
                            hT[:, rt, :], pst[rt][:],
                            ACTF.Copy,
                            scale=s_comb[:, rt:rt + 1])

                # ---- mm2: y[tok, o] ----
                for t in range(TBS // P):           # 4 token sub-tiles
                    for ob in range(NOB):           # 8 blocks of 512 outputs
                        psy = psp.tile([P, 512], f32, tag="ps", name="ps2")
                        for rk in range(RO):
                            nc.tensor.matmul(
                                psy[:],
                                hT[:, rk, t * P:(t + 1) * P],
                                u_t[rk][:, ob * 512:(ob + 1) * 512],
                                start=(rk == 0), stop=(rk == RO - 1))
                        r0 = tok0 + t * P
                        yt = yout.tile([P, 512], bf16, name="yt")
                        bsl = bias_sb[:, ob * 512:(ob + 1) * 512]
                        last = (blk == TB - 1 and t == TBS // P - 1
                                and ob == NOB - 1)
                        if last:
                            # split the final tile in half so its DMA can
                            # start early and drain on two queues at once
                            # (GpSimd's queue is idle at the end; Sync's
                            # carries the y backlog and drains late)
                            nc.vector.tensor_tensor(
                                yt[:, 0:256], psy[:, 0:256],
                                bias_sb[:, ob * 512:ob * 512 + 256], OP.add)
                            nc.scalar.dma_start(
                                y.ap()[r0:r0 + P, ob * 512:ob * 512 + 256],
                                yt[:, 0:256])
                            nc.vector.tensor_tensor(
                                yt[:, 256:512], psy[:, 256:512],
                                bias_sb[:, ob * 512 + 256:(ob + 1) * 512],
                                OP.add)
                            nc.gpsimd.dma_start(
                                y.ap()[r0:r0 + P,
                                       ob * 512 + 256:(ob + 1) * 512],
                                yt[:, 256:512])
                        else:
                            nc.vector.tensor_tensor(yt[:], psy[:], bsl,
                                                    OP.add)
                            if (t * NOB + ob) % 2 == 0:
                                nc.scalar.dma_start(
                                    y.ap()[r0:r0 + P,
                                           ob * 512:(ob + 1) * 512],
                                    yt[:])
                            else:
                                nc.gpsimd.dma_start(
                                    y.ap()[r0:r0 + P,
                                           ob * 512:(ob + 1) * 512],
                                    yt[:])

    nc.compile()
    return nc


def _maybe_enable_trace():
    """Register the axon NTFF profile hook (test/dev only, KERNEL_TRACE=1)."""
    try:
        import sys
        import types

        try:
            from antenv.axon_hooks import get_axon_ntff_profile_hook  # noqa: F401
        except ImportError:
            store = {"h": None}
            mod = types.ModuleType("antenv.axon_hooks")
            mod.set_axon_ntff_profile_hook = lambda h: store.__setitem__("h", h)
            mod.get_axon_ntff_profile_hook = lambda: store["h"]
            sys.modules["antenv.axon_hooks"] = mod
        from antenv.axon_hooks import set_axon_ntff_profile_hook
        from trn_agent_boot.trn_boot import _ntff_profile_via_ctypes

        set_axon_ntff_profile_hook(
            _ntff_profile_via_ctypes("/opt/axon/libaxon_pjrt.so"))
        import concourse.bass_utils as bass_utils

        bass_utils.upload_artifacts = lambda tmpdir: tmpdir
        return True
    except Exception as e:  # pragma: no cover - trace is best-effort
        print(f"trace setup failed: {e}")
        return False


def kernel(x, U_data, U_scale, U_zp, S_data, S_scale, S_zp,
           Vh_data, Vh_scale, Vh_zp, bias):
    global last_run

    trace = bool(os.environ.get("KERNEL_TRACE"))
    if trace:
        trace = _maybe_enable_trace()

    from concourse.bass_utils import run_bass_kernel_spmd

    x = np.asarray(x, dtype=np.float32)
    bias_np = np.asarray(bias, dtype=np.float32)
    s_v = float(np.asarray(Vh_scale).reshape(-1)[0])
    s_u = float(np.asarray(U_scale).reshape(-1)[0])
    s_s = float(np.asarray(S_scale).reshape(-1)[0])
    zp_v = float(np.asarray(Vh_zp).reshape(-1)[0])
    zp_u = float(np.asarray(U_zp).reshape(-1)[0])
    zp_s = float(np.asarray(S_zp).reshape(-1)[0])

    # Host: shard x, cast to the kernel's bf16 compute precision, and repack
    # so every DMA partition line is 4 KiB contiguous. Weights get a lossless
    # int32->bf16 cast (values are 0..255, exact in bf16).
    x_bf = x.reshape(NTOK, IN_F).astype(_BF16)
    vhT = np.ascontiguousarray(np.asarray(Vh_data).T).astype(np.uint8)
    uT = np.ascontiguousarray(np.asarray(U_data).T).astype(np.uint8)
    sv = np.asarray(S_data).astype(_BF16)                          # [1024]

    key = (zp_v, zp_u, zp_s, s_v * s_s * s_u)
    nc = _nc_cache.get(key)
    if nc is None:
        nc = _nc_cache[key] = _build_nc(*key)

    in_maps = []
    for c in range(N_CORES):
        xc = x_bf[c * TPC:(c + 1) * TPC]                           # [1024, 4096]
        # xr[blk, ko4, p, four*512+t] = xc[blk*512+t, ko4*512+four*128+p]
        xrc = np.ascontiguousarray(
            xc.reshape(TB, TBS, KO // 4, 4, P).transpose(0, 2, 4, 3, 1)
        ).reshape(TB, KO // 4, P, 4 * TBS)
        in_maps.append({
            "xr": xrc,
            "vhT": vhT,
            "uT": uT,
            "sv": sv,
            "bias": bias_np,
        })

    kwargs = {}
    if trace:
        kwargs = dict(trace=True, tmpdir=os.environ.get("KERNEL_TRACE_DIR"))
        if os.environ.get("KERNEL_TRACE_ALL"):
            kwargs["trace_cores"] = list(range(N_CORES))
    res = run_bass_kernel_spmd(nc, in_maps, core_ids=list(range(N_CORES)),
                               **kwargs)
    last_run = res

    y = np.concatenate(
        [res.results[c]["y"] for c in range(N_CORES)], axis=0
    ).astype(np.float32)
    return y.reshape(B, S_LEN, OUT_F)

